# revision 3
# baseline (speedup 1.0000x reference)
"""CGRU cell on 8 Trainium2 NeuronCores.

Strategy: data-parallel over the batch dim (4096 -> 8 x 512). Each core
computes its h-shard with zero cross-core communication; weights are
replicated.

On-core compute runs in transposed space ([feature, batch]): the gate
pre-activations are (x @ W)^T = W^T @ x^T, so W tiles are the stationary
matmul operand and x^T/h^T tiles [128, 512] are the moving operand
(batch = 512 = one full fp32 PSUM bank).  The complex "cat kernel"
[[R, -I], [I, R]] is never materialized: real-out = R^Txr + I^Txi,
imag-out = R^Txi + (-I)^Txr, with R tiles shared by both outputs and
pre-negated I copies built on the host.  All matmuls are fp16 with fp32
PSUM accumulation; the final combine (z*h + (1-z)*hh) is fp32.
"""

import numpy as np

import concourse.bass as bass
import concourse.mybir as mybir
import concourse.tile as tile
from concourse import bacc
from concourse.bass_utils import run_bass_kernel_spmd

B, D, U = 4096, 1024, 1024
NCORES = 8
N = B // NCORES          # batch rows per core (moving free dim)
P = 128                  # partition size
KT = D // P              # 8 k-tiles per complex half
MT = U // P              # 8 m-tiles per complex half
F = 2 * U                # 2048 features
NMAT = 3                 # mats per blob half (R, I, -I) / (RR, IR, -IR)
HCOLS = NMAT * KT * P    # 3072 cols per weight half-tile

F16 = mybir.dt.float16
F32 = mybir.dt.float32
AF = mybir.ActivationFunctionType
OP = mybir.AluOpType

_CACHE = {}


def _build():
    nc = bacc.Bacc("TRN2", target_bir_lowering=False, debug=False)

    xT = nc.dram_tensor("xT", [F, N], F16, kind="ExternalInput")
    hTf = nc.dram_tensor("hTf", [F, N], F32, kind="ExternalInput")
    w1 = nc.dram_tensor("w1", [MT, 2, 2, P, HCOLS], F16, kind="ExternalInput")
    w2 = nc.dram_tensor("w2", [MT, 2, P, HCOLS], F16, kind="ExternalInput")
    bzr = nc.dram_tensor("bzr", [2, 2 * MT, P], F32, kind="ExternalInput")
    bh = nc.dram_tensor("bh", [2 * MT, P], F32, kind="ExternalInput")
    oT = nc.dram_tensor("oT", [F, N], F32, kind="ExternalOutput")

    with tile.TileContext(nc) as tc:
        with (
            tc.tile_pool(name="res", bufs=1) as res,
            tc.tile_pool(name="wts", bufs=3) as wts,
            tc.tile_pool(name="act", bufs=3) as act,
            tc.tile_pool(name="ps", bufs=6, space="PSUM") as psp,
        ):
            xs = res.tile([P, 2 * MT, N], F16, tag="xs")
            hs = res.tile([P, 2 * MT, N], F16, tag="hs")
            hs32 = res.tile([P, 2 * MT, N], F32, tag="hs32")
            zs = res.tile([P, 2 * MT, N], F16, tag="zs")
            rh = res.tile([P, 2 * MT, N], F16, tag="rh")
            bz_sb = res.tile([P, 2, 2 * MT], F32, tag="bz")
            bh_sb = res.tile([P, 2 * MT], F32, tag="bh")

            nc.sync.dma_start(bz_sb[:], bzr.rearrange("g m p -> p g m"))
            nc.sync.dma_start(bh_sb[:], bh.rearrange("m p -> p m"))
            for j in range(2 * MT):
                nc.sync.dma_start(xs[:, j, :], xT[j * P:(j + 1) * P, :])
                nc.sync.dma_start(hs32[:, j, :], hTf[j * P:(j + 1) * P, :])
                nc.vector.tensor_copy(hs[:, j, :], hs32[:, j, :])

            def accum_pair(wt_a, wt_b, movs_a, movs_b, ps_r, ps_i):
                """64 matmuls: two psum tiles (real/imag out halves).

                Each half-tile holds mats [S, A, B] x k: S is shared by
                both outputs, A feeds only real-out, B only imag-out.
                """
                for wt, (mov_s_r, mov_s_i, mov_a, mov_b) in (
                    (wt_a, movs_a), (wt_b, movs_b),
                ):
                    first = wt is wt_a
                    for k in range(KT):
                        wap = wt[:, (0 * KT + k) * P:(0 * KT + k + 1) * P]
                        nc.tensor.matmul(ps_r[:], wap, mov_s_r(k),
                                         start=(first and k == 0), stop=False)
                        nc.tensor.matmul(ps_i[:], wap, mov_s_i(k),
                                         start=(first and k == 0), stop=False)
                    for k in range(KT):
                        wap = wt[:, (1 * KT + k) * P:(1 * KT + k + 1) * P]
                        nc.tensor.matmul(ps_r[:], wap, mov_a(k),
                                         start=False, stop=(not first and k == KT - 1))
                    for k in range(KT):
                        wap = wt[:, (2 * KT + k) * P:(2 * KT + k + 1) * P]
                        nc.tensor.matmul(ps_i[:], wap, mov_b(k),
                                         start=False, stop=(not first and k == KT - 1))

            def xr(k):
                return xs[:, k, :]

            def xi(k):
                return xs[:, MT + k, :]

            def hr(k):
                return hs[:, k, :]

            def hi(k):
                return hs[:, MT + k, :]

            def rhr(k):
                return rh[:, k, :]

            def rhi(k):
                return rh[:, MT + k, :]

            # movs per half-tile: (shared->real, shared->imag, real-only, imag-only)
            movs_x = (xr, xi, xi, xr)
            movs_h = (hr, hi, hi, hr)
            movs_rh = (rhr, rhi, rhi, rhr)

            # ---- phase 1: z and r gates, rh = clip(r)*h ----
            for p in range(MT):
                for g in range(2):
                    wt_a = wts.tile([P, HCOLS], F16, tag="w")
                    wt_b = wts.tile([P, HCOLS], F16, tag="w")
                    nc.sync.dma_start(wt_a[:], w1[p, g, 0])
                    nc.sync.dma_start(wt_b[:], w1[p, g, 1])
                    ps_r = psp.tile([P, N], F32, tag="ps")
                    ps_i = psp.tile([P, N], F32, tag="ps")
                    accum_pair(wt_a, wt_b, movs_x, movs_h, ps_r, ps_i)
                    if g == 0:
                        nc.scalar.activation(zs[:, p, :], ps_r[:], AF.Relu,
                                             bias=bz_sb[:, 0, p:p + 1], scale=0.2)
                        nc.scalar.activation(zs[:, MT + p, :], ps_i[:], AF.Relu,
                                             bias=bz_sb[:, 0, MT + p:MT + p + 1],
                                             scale=0.2)
                    else:
                        rr = act.tile([P, N], F16, tag="rr")
                        ri = act.tile([P, N], F16, tag="rr")
                        nc.scalar.activation(rr[:], ps_r[:], AF.Relu,
                                             bias=bz_sb[:, 1, p:p + 1], scale=0.2)
                        nc.scalar.activation(ri[:], ps_i[:], AF.Relu,
                                             bias=bz_sb[:, 1, MT + p:MT + p + 1],
                                             scale=0.2)
                        nc.vector.scalar_tensor_tensor(
                            rh[:, p, :], rr[:], 1.0, hs[:, p, :],
                            op0=OP.min, op1=OP.mult)
                        nc.vector.scalar_tensor_tensor(
                            rh[:, MT + p, :], ri[:], 1.0, hs[:, MT + p, :],
                            op0=OP.min, op1=OP.mult)

            # ---- phase 2: hh gate + final combine ----
            for p in range(MT):
                wt_a = wts.tile([P, HCOLS], F16, tag="w")
                wt_b = wts.tile([P, HCOLS], F16, tag="w")
                nc.sync.dma_start(wt_a[:], w2[p, 0])
                nc.sync.dma_start(wt_b[:], w2[p, 1])
                ps_r = psp.tile([P, N], F32, tag="ps")
                ps_i = psp.tile([P, N], F32, tag="ps")
                accum_pair(wt_a, wt_b, movs_x, movs_rh, ps_r, ps_i)
                for m, ps in ((p, ps_r), (MT + p, ps_i)):
                    t = act.tile([P, N], F32, tag="t")
                    nc.scalar.activation(t[:], ps[:], AF.Tanh,
                                         bias=bh_sb[:, m:m + 1])
                    zmin = act.tile([P, N], F32, tag="zm")
                    nc.vector.tensor_scalar_min(zmin[:], zs[:, m, :], 1.0)
                    d = act.tile([P, N], F32, tag="d")
                    nc.vector.scalar_tensor_tensor(
                        d[:], t[:], -1.0, hs32[:, m, :],
                        op0=OP.mult, op1=OP.add)
                    e = act.tile([P, N], F32, tag="e")
                    nc.vector.tensor_tensor(e[:], zmin[:], d[:], OP.mult)
                    o = act.tile([P, N], F32, tag="o")
                    nc.vector.tensor_tensor(o[:], e[:], t[:], OP.add)
                    nc.sync.dma_start(oT[m * P:(m + 1) * P, :], o[:])

    nc.compile()
    return nc


def _tiles(mat):
    # (1024, 1024) -> [p, k, 128, 128] tile array
    return mat.reshape(KT, P, MT, P).transpose(2, 0, 1, 3)


def _gate_blob(mats_a, mats_b):
    """[p, half, 128, HCOLS] fp16 weight blob from two triples of mats."""
    halves = []
    for mats in (mats_a, mats_b):
        arr = np.stack([_tiles(m) for m in mats])       # [3, p, k, 128, 128]
        arr = arr.transpose(1, 3, 0, 2, 4)              # [p, part, mat, k, col]
        halves.append(arr.reshape(MT, P, HCOLS))
    return np.stack(halves, axis=1).astype(np.float16)  # [p, 2, 128, HCOLS]


def prepare_in_maps(inputs, h_tm1, real_kernel, imaginary_kernel,
                    real_recurrent_kernel, imaginary_recurrent_kernel,
                    real_bias, imaginary_bias):
    inputs = np.asarray(inputs, dtype=np.float32)
    h_tm1 = np.asarray(h_tm1, dtype=np.float32)

    def gate(Wmat, g):
        return np.asarray(Wmat[:, g * U:(g + 1) * U], dtype=np.float32)

    blobs1 = []  # per gate z, r: (p, 2, 128, HCOLS)
    for g in range(2):
        R, I = gate(real_kernel, g), gate(imaginary_kernel, g)
        RR, IR = gate(real_recurrent_kernel, g), gate(imaginary_recurrent_kernel, g)
        blobs1.append(_gate_blob((R, I, -I), (RR, IR, -IR)))
    w1_np = np.ascontiguousarray(np.stack(blobs1, axis=1))  # (p, 2, 2, 128, HCOLS)
    R, I = gate(real_kernel, 2), gate(imaginary_kernel, 2)
    RR, IR = gate(real_recurrent_kernel, 2), gate(imaginary_recurrent_kernel, 2)
    w2_np = np.ascontiguousarray(_gate_blob((R, I, -I), (RR, IR, -IR)))

    def cat_bias(g):
        return np.concatenate([
            np.asarray(real_bias[g * U:(g + 1) * U], dtype=np.float32),
            np.asarray(imaginary_bias[g * U:(g + 1) * U], dtype=np.float32),
        ])

    bzr_np = np.ascontiguousarray(np.stack(
        [0.2 * cat_bias(g) + 0.5 for g in range(2)]).reshape(2, 2 * MT, P))
    bh_np = np.ascontiguousarray(cat_bias(2).reshape(2 * MT, P))

    in_maps = []
    for c in range(NCORES):
        sl = slice(c * N, (c + 1) * N)
        in_maps.append({
            "xT": inputs[sl].T.astype(np.float16),
            "hTf": np.ascontiguousarray(h_tm1[sl].T),
            "w1": w1_np, "w2": w2_np, "bzr": bzr_np, "bh": bh_np,
        })
    return in_maps


def get_nc():
    if "nc" not in _CACHE:
        _CACHE["nc"] = _build()
    return _CACHE["nc"]


def gather(results):
    out = np.empty((B, F), dtype=np.float32)
    for c in range(NCORES):
        out[c * N:(c + 1) * N] = results[c]["oT"].T
    return out


def kernel(**inputs):
    nc = get_nc()
    in_maps = prepare_in_maps(**inputs)
    res = run_bass_kernel_spmd(nc, in_maps, list(range(NCORES)))
    return gather(res.results)


# revision 5
# speedup vs baseline: 1.0485x; 1.0485x over previous
"""CGRU cell on 8 Trainium2 NeuronCores.

Strategy: data-parallel over the batch dim (4096 -> 8 x 512). Each core
computes its h-shard with zero cross-core communication; weights are
replicated.

On-core compute runs in transposed space ([feature, batch]): the gate
pre-activations are (x @ W)^T = W^T @ x^T, so W tiles are the stationary
matmul operand and x^T/h^T tiles [128, 512] are the moving operand
(batch = 512 = one full fp32 PSUM bank).  The complex "cat kernel"
[[R, -I], [I, R]] is never materialized: real-out = R^Txr + I^Txi,
imag-out = R^Txi + (-I)^Txr, with R tiles shared by both outputs and
pre-negated I copies built on the host.  All matmuls are fp16 with fp32
PSUM accumulation; the final combine (z*h + (1-z)*hh) is fp32.

DMA queues: weights stream on the sync queue, activations load on the
gpsimd queue, outputs store on the scalar queue, so the first weight
tile is not stuck behind 6MB of activation loads.
"""

import numpy as np

import concourse.bass as bass
import concourse.mybir as mybir
import concourse.tile as tile
from concourse import bacc
from concourse.bass_utils import run_bass_kernel_spmd

B, D, U = 4096, 1024, 1024
NCORES = 8
N = B // NCORES          # batch rows per core (moving free dim)
P = 128                  # partition size
KT = D // P              # 8 k-tiles per complex half
MT = U // P              # 8 m-tiles per complex half
F = 2 * U                # 2048 features
MCOLS = KT * P           # 1024 cols per per-matrix weight tile

F16 = mybir.dt.float16
F32 = mybir.dt.float32
AF = mybir.ActivationFunctionType
OP = mybir.AluOpType

_CACHE = {}


def _build():
    nc = bacc.Bacc("TRN2", target_bir_lowering=False, debug=False)

    xT = nc.dram_tensor("xT", [F, N], F16, kind="ExternalInput")
    hT16 = nc.dram_tensor("hT16", [F, N], F16, kind="ExternalInput")
    hTf = nc.dram_tensor("hTf", [F, N], F32, kind="ExternalInput")
    w1 = nc.dram_tensor("w1", [MT, 2, 6, P, MCOLS], F16, kind="ExternalInput")
    w2 = nc.dram_tensor("w2", [MT, 6, P, MCOLS], F16, kind="ExternalInput")
    bzr = nc.dram_tensor("bzr", [2, 2 * MT, P], F32, kind="ExternalInput")
    bh = nc.dram_tensor("bh", [2 * MT, P], F32, kind="ExternalInput")
    oT = nc.dram_tensor("oT", [F, N], F32, kind="ExternalOutput")

    with tile.TileContext(nc) as tc:
        with (
            tc.tile_pool(name="res", bufs=1) as res,
            tc.tile_pool(name="wts", bufs=9) as wts,
            tc.tile_pool(name="act", bufs=3) as act,
            tc.tile_pool(name="ps", bufs=6, space="PSUM") as psp,
            tc.tile_pool(name="wm", bufs=1, space="PSUM") as wmp,
        ):
            # PE warmup: dummy matmuls on a zeroed tile keep the HAM
            # activity window busy while the first real DMAs land.
            wsrc = res.tile([P, P], F16, tag="wsrc")
            dmov = res.tile([P, N], F16, tag="dmov")
            nc.gpsimd.memset(wsrc[:], 0.0)
            nc.gpsimd.memset(dmov[:], 0.0)
            wps = wmp.tile([P, N], F32, tag="warm")
            for _ in range(20):
                nc.tensor.matmul(wps[:], wsrc[:], dmov[:], start=True, stop=True)

            xs = res.tile([P, 2 * MT, N], F16, tag="xs")
            hs = res.tile([P, 2 * MT, N], F16, tag="hs")
            hs32 = res.tile([P, 2 * MT, N], F32, tag="hs32")
            zs = res.tile([P, 2 * MT, N], F16, tag="zs")
            rh = res.tile([P, 2 * MT, N], F16, tag="rh")
            bz_sb = res.tile([P, 2, 2 * MT], F32, tag="bz")
            bh_sb = res.tile([P, 2 * MT], F32, tag="bh")

            for j in range(2 * MT):
                nc.gpsimd.dma_start(xs[:, j, :], xT[j * P:(j + 1) * P, :])
            for j in range(2 * MT):
                nc.gpsimd.dma_start(hs[:, j, :], hT16[j * P:(j + 1) * P, :])
            nc.gpsimd.dma_start(bz_sb[:], bzr.rearrange("g m p -> p g m"))
            nc.gpsimd.dma_start(bh_sb[:], bh.rearrange("m p -> p m"))
            for j in range(2 * MT):
                nc.gpsimd.dma_start(hs32[:, j, :], hTf[j * P:(j + 1) * P, :])

            def load_w(src):
                """six [P, MCOLS] weight tiles: R, I, -I, RR, IR, -IR"""
                ws = []
                for i in range(6):
                    wt = wts.tile([P, MCOLS], F16, tag="w")
                    nc.sync.dma_start(wt[:], src[i])
                    ws.append(wt)
                return ws

            def accum_pair(ws, movs_a, movs_b, ps_r, ps_i):
                """64 matmuls into a (real-out, imag-out) psum pair."""
                for h, (sh_r, sh_i, only_r, only_i) in ((0, movs_a), (3, movs_b)):
                    for k in range(KT):
                        wap = ws[h][:, k * P:(k + 1) * P]
                        nc.tensor.matmul(ps_r[:], wap, sh_r(k),
                                         start=(h == 0 and k == 0), stop=False)
                        nc.tensor.matmul(ps_i[:], wap, sh_i(k),
                                         start=(h == 0 and k == 0), stop=False)
                    for k in range(KT):
                        nc.tensor.matmul(ps_r[:], ws[h + 1][:, k * P:(k + 1) * P],
                                         only_r(k),
                                         start=False, stop=(h == 3 and k == KT - 1))
                    for k in range(KT):
                        nc.tensor.matmul(ps_i[:], ws[h + 2][:, k * P:(k + 1) * P],
                                         only_i(k),
                                         start=False, stop=(h == 3 and k == KT - 1))

            def xr(k):
                return xs[:, k, :]

            def xi(k):
                return xs[:, MT + k, :]

            def hr(k):
                return hs[:, k, :]

            def hi(k):
                return hs[:, MT + k, :]

            def rhr(k):
                return rh[:, k, :]

            def rhi(k):
                return rh[:, MT + k, :]

            # movs per weight triple: (shared->real, shared->imag,
            # I->real-only, -I->imag-only)
            movs_x = (xr, xi, xi, xr)
            movs_h = (hr, hi, hi, hr)
            movs_rh = (rhr, rhi, rhi, rhr)

            # ---- phase 1: z and r gates, rh = clip(r)*h ----
            for p in range(MT):
                for g in range(2):
                    ws = load_w(w1[p, g])
                    ps_r = psp.tile([P, N], F32, tag="ps")
                    ps_i = psp.tile([P, N], F32, tag="ps")
                    accum_pair(ws, movs_x, movs_h, ps_r, ps_i)
                    if g == 0:
                        nc.scalar.activation(zs[:, p, :], ps_r[:], AF.Relu,
                                             bias=bz_sb[:, 0, p:p + 1], scale=0.2)
                        nc.scalar.activation(zs[:, MT + p, :], ps_i[:], AF.Relu,
                                             bias=bz_sb[:, 0, MT + p:MT + p + 1],
                                             scale=0.2)
                    else:
                        rr = act.tile([P, N], F16, tag="rr")
                        ri = act.tile([P, N], F16, tag="rr")
                        nc.scalar.activation(rr[:], ps_r[:], AF.Relu,
                                             bias=bz_sb[:, 1, p:p + 1], scale=0.2)
                        nc.scalar.activation(ri[:], ps_i[:], AF.Relu,
                                             bias=bz_sb[:, 1, MT + p:MT + p + 1],
                                             scale=0.2)
                        nc.vector.scalar_tensor_tensor(
                            rh[:, p, :], rr[:], 1.0, hs[:, p, :],
                            op0=OP.min, op1=OP.mult)
                        nc.vector.scalar_tensor_tensor(
                            rh[:, MT + p, :], ri[:], 1.0, hs[:, MT + p, :],
                            op0=OP.min, op1=OP.mult)

            # ---- phase 2: hh gate + final combine ----
            for p in range(MT):
                ws = load_w(w2[p])
                ps_r = psp.tile([P, N], F32, tag="ps")
                ps_i = psp.tile([P, N], F32, tag="ps")
                accum_pair(ws, movs_x, movs_rh, ps_r, ps_i)
                for m, ps in ((p, ps_r), (MT + p, ps_i)):
                    t = act.tile([P, N], F32, tag="t")
                    nc.scalar.activation(t[:], ps[:], AF.Tanh,
                                         bias=bh_sb[:, m:m + 1])
                    zmin = act.tile([P, N], F32, tag="zm")
                    nc.vector.tensor_scalar_min(zmin[:], zs[:, m, :], 1.0)
                    d = act.tile([P, N], F32, tag="d")
                    nc.vector.scalar_tensor_tensor(
                        d[:], t[:], -1.0, hs32[:, m, :],
                        op0=OP.mult, op1=OP.add)
                    e = act.tile([P, N], F32, tag="e")
                    nc.vector.tensor_tensor(e[:], zmin[:], d[:], OP.mult)
                    o = act.tile([P, N], F32, tag="o")
                    nc.vector.tensor_tensor(o[:], e[:], t[:], OP.add)
                    nc.scalar.dma_start(oT[m * P:(m + 1) * P, :], o[:])

    nc.compile()
    return nc


def _tiles(mat):
    # (1024, 1024) -> [p, k, 128, 128] tile array
    return mat.reshape(KT, P, MT, P).transpose(2, 0, 1, 3)


def _gate_blob(mats):
    """[p, 6, 128, MCOLS] fp16 weight blob from (R, I, -I, RR, IR, -IR)."""
    arr = np.stack([_tiles(m) for m in mats])  # [6, p, k, part, col]
    arr = arr.transpose(1, 0, 3, 2, 4)         # [p, mat, part, k, col]
    return arr.reshape(MT, 6, P, MCOLS).astype(np.float16)


def prepare_in_maps(inputs, h_tm1, real_kernel, imaginary_kernel,
                    real_recurrent_kernel, imaginary_recurrent_kernel,
                    real_bias, imaginary_bias):
    inputs = np.asarray(inputs, dtype=np.float32)
    h_tm1 = np.asarray(h_tm1, dtype=np.float32)

    def gate(Wmat, g):
        return np.asarray(Wmat[:, g * U:(g + 1) * U], dtype=np.float32)

    def mats(g):
        R, I = gate(real_kernel, g), gate(imaginary_kernel, g)
        RR, IR = gate(real_recurrent_kernel, g), gate(imaginary_recurrent_kernel, g)
        return (R, I, -I, RR, IR, -IR)

    w1_np = np.ascontiguousarray(
        np.stack([_gate_blob(mats(0)), _gate_blob(mats(1))], axis=1))
    w2_np = np.ascontiguousarray(_gate_blob(mats(2)))

    def cat_bias(g):
        return np.concatenate([
            np.asarray(real_bias[g * U:(g + 1) * U], dtype=np.float32),
            np.asarray(imaginary_bias[g * U:(g + 1) * U], dtype=np.float32),
        ])

    bzr_np = np.ascontiguousarray(np.stack(
        [0.2 * cat_bias(g) + 0.5 for g in range(2)]).reshape(2, 2 * MT, P))
    bh_np = np.ascontiguousarray(cat_bias(2).reshape(2 * MT, P))

    in_maps = []
    for c in range(NCORES):
        sl = slice(c * N, (c + 1) * N)
        hT = np.ascontiguousarray(h_tm1[sl].T)
        in_maps.append({
            "xT": inputs[sl].T.astype(np.float16),
            "hT16": hT.astype(np.float16),
            "hTf": hT,
            "w1": w1_np, "w2": w2_np, "bzr": bzr_np, "bh": bh_np,
        })
    return in_maps


def get_nc():
    if "nc" not in _CACHE:
        _CACHE["nc"] = _build()
    return _CACHE["nc"]


def gather(results):
    out = np.empty((B, F), dtype=np.float32)
    for c in range(NCORES):
        out[c * N:(c + 1) * N] = res_oT(results, c)
    return out


def res_oT(results, c):
    return results[c]["oT"].T


def kernel(**inputs):
    nc = get_nc()
    in_maps = prepare_in_maps(**inputs)
    res = run_bass_kernel_spmd(nc, in_maps, list(range(NCORES)))
    return gather(res.results)
